# revision 26
# baseline (speedup 1.0000x reference)
"""MeshFC kernel for 8x TRN2 NeuronCores.

Computes: out = inputs @ w + biases, where
  w[i,o] = ||in_pos[i]-out_pos[o]|| - ||init_in_pos[i]-init_out_pos[o]||

Sharding: tensor-parallel on the output dim (8 x 1024 columns). Each core:
  - generates its weight column block on-chip via the PE using the
    augmented-inner-product identity dist^2 = ||a||^2 - 2 a.b + ||b||^2
    (a single K=7 fp32 matmul per tile), sqrt on ScalarE, subtract on DVE
  - runs the main [4096,2048]x[2048,1024] matmul in float32r (FP22)
Host side: pre-transposes/pre-tiles inputs so every DMA is contiguous,
and concatenates the 8 per-core [4096,1024] outputs.
"""

import os
from contextlib import ExitStack

import numpy as np

NUM_IN, NUM_OUT, SD, BATCH = 2048, 8192, 5, 4096
N_CORES = 8
O_SHARD = NUM_OUT // N_CORES  # 1024
B_TILES = BATCH // 128  # 32
K_TILES = NUM_IN // 128  # 16
O_HALves = O_SHARD // 512  # 2

_CACHE = {}


def _build_bass(variant=""):
    import concourse.bass as bass  # noqa: F401
    import concourse.mybir as mybir
    from concourse import bacc
    from concourse.tile import TileContext

    fp32 = mybir.dt.float32
    fp32r = mybir.dt.float32r

    # Bacc (not plain Bass): its compile() runs generate_event_semaphores +
    # move_matmul_waits_to_ldweights, which split multi-waits that exceed the
    # per-instruction HW sync-wait budget.
    nc = bacc.Bacc("TRN2", name="meshfc")

    xT = nc.dram_tensor("xT", [B_TILES, 128, NUM_IN], fp32r, kind="ExternalInput")
    # packed [aC | aI | bC | bI] along the free axis -> single DMA, single wait
    AB_W = 2 * NUM_IN + 2 * O_SHARD
    ab = nc.dram_tensor("ab", [7, AB_W], fp32, kind="ExternalInput")
    # [bias | ones(128)] packed on one partition
    bias = nc.dram_tensor("bias", [1, O_SHARD + 128], fp32r, kind="ExternalInput")
    out = nc.dram_tensor("out", [BATCH, O_SHARD], fp32, kind="ExternalOutput")

    with ExitStack() as ctx:
        tc = ctx.enter_context(TileContext(nc))
        const = ctx.enter_context(tc.tile_pool(name="const", bufs=1))
        wps = ctx.enter_context(tc.tile_pool(name="wps", bufs=2, space="PSUM"))
        mps = ctx.enter_context(tc.tile_pool(name="mps", bufs=2, space="PSUM"))
        tmp = ctx.enter_context(tc.tile_pool(name="tmp", bufs=2))
        xpool = ctx.enter_context(tc.tile_pool(name="xp", bufs=3))
        opool = ctx.enter_context(tc.tile_pool(name="op", bufs=3))

        # --- constants ---
        ab_sb = const.tile([7, AB_W], fp32, name="ab_sb")
        nc.sync.dma_start(out=ab_sb, in_=ab[:, :])
        aC_sb = ab_sb[:, 0:NUM_IN]
        aI_sb = ab_sb[:, NUM_IN : 2 * NUM_IN]
        bC_sb = ab_sb[:, 2 * NUM_IN : 2 * NUM_IN + O_SHARD]
        bI_sb = ab_sb[:, 2 * NUM_IN + O_SHARD : AB_W]

        # bias lives on one partition; it is added into PSUM via a K=1 matmul
        biasones_sb = const.tile([1, O_SHARD + 128], fp32r, name="biasones_sb")
        nc.sync.dma_start(out=biasones_sb, in_=bias[:, :])
        bias_sb = biasones_sb[:, 0:O_SHARD]
        ones_sb = biasones_sb[:, O_SHARD : O_SHARD + 128]

        # resident weight block: [128, K_TILES, O_SHARD] = 8 MB
        # float32r so the DVE write rounds to FP22 for the fp32r matmul
        w_sb = const.tile([128, K_TILES, O_SHARD], fp32r, name="w_sb")

        # --- weight generation ---
        if "nowgen" not in variant:
            for kt in range(K_TILES):
                for oh in range(O_HALves):
                    osl = slice(oh * 512, (oh + 1) * 512)
                    psC = wps.tile([128, 512], fp32, tag="psC", bufs=2)
                    psI = wps.tile([128, 512], fp32, tag="psI", bufs=2)
                    nc.tensor.matmul(
                        psC,
                        aC_sb[:, kt * 128 : (kt + 1) * 128],
                        bC_sb[:, osl],
                        start=True,
                        stop=True,
                    )
                    nc.tensor.matmul(
                        psI,
                        aI_sb[:, kt * 128 : (kt + 1) * 128],
                        bI_sb[:, osl],
                        start=True,
                        stop=True,
                    )
                    # clamp dist^2 to >=0 on DVE (HW fp32 rounding can push
                    # the closest pair slightly negative -> sqrt NaN), then
                    # sqrt in place in SBUF. In-place PSUM activation crashes
                    # the exec unit, so everything lands in SBUF tmps.
                    sC = tmp.tile([128, 512], fp32, tag="sC", bufs=2)
                    sI = tmp.tile([128, 512], fp32, tag="sI", bufs=2)
                    nc.vector.tensor_scalar_max(sC, psC, 0.0)
                    nc.vector.tensor_scalar_max(sI, psI, 0.0)
                    nc.scalar.sqrt(sC, sC)
                    nc.scalar.sqrt(sI, sI)
                    nc.vector.tensor_sub(w_sb[:, kt, osl], sC, sI)

        # --- main matmul: out[b,o] = sum_k x[b,k] w[k,o] (+bias) ---
        for bt in range(B_TILES):
            xt = xpool.tile([128, NUM_IN], fp32r, name="xt")
            nc.sync.dma_start(out=xt, in_=xT[bt])
            ot = opool.tile([128, O_SHARD], fp32, name="ot")
            # pre-touch: absorbs the out-DMA slot-release wait on ScalarE so
            # the real drains below stay within the HW sync-wait slot limit
            nc.scalar.mul(ot[0:1, 0:1], ot[0:1, 0:1], 0.0)
            for oh in range(O_HALves):
                osl = slice(oh * 512, (oh + 1) * 512)
                ps = mps.tile([128, 512], fp32, tag="ps", bufs=2)
                for kt in range(K_TILES):
                    nc.tensor.matmul(
                        ps,
                        xt[:, kt * 128 : (kt + 1) * 128],
                        w_sb[:, kt, osl],
                        start=(kt == 0),
                        stop=("nobias" in variant and kt == K_TILES - 1),
                    )
                # += bias (broadcast over rows via rank-1 matmul)
                if "nobias" not in variant:
                    nc.tensor.matmul(
                        ps, ones_sb[:, :], bias_sb[:, osl], start=False, stop=True
                    )
                nc.scalar.copy(ot[:, osl], ps)
            nc.sync.dma_start(out=out[bt * 128 : (bt + 1) * 128, :], in_=ot)

    nc.finalize()
    return nc


def _prep_inputs(inputs, init_in_pos, init_out_pos, in_pos, out_pos, biases):
    x = np.ascontiguousarray(np.asarray(inputs, dtype=np.float32))
    a = np.asarray(in_pos, dtype=np.float32).reshape(NUM_IN, SD)
    a0 = np.asarray(init_in_pos, dtype=np.float32).reshape(NUM_IN, SD)
    b = np.asarray(out_pos, dtype=np.float32).reshape(NUM_OUT, SD)
    b0 = np.asarray(init_out_pos, dtype=np.float32).reshape(NUM_OUT, SD)
    bias = np.asarray(biases, dtype=np.float32).reshape(NUM_OUT)

    # [bt, p, kt*128+b'] = x[bt*128+b', kt*128+p]
    xT = np.ascontiguousarray(
        x.reshape(B_TILES, 128, K_TILES, 128).transpose(0, 3, 2, 1)
    ).reshape(B_TILES, 128, NUM_IN)

    def aug_a(p):
        return np.concatenate(
            [p.T, (p * p).sum(1)[None, :], np.ones((1, p.shape[0]), np.float32)], 0
        ).astype(np.float32)

    def aug_b(q):
        return np.concatenate(
            [-2.0 * q.T, np.ones((1, q.shape[0]), np.float32), (q * q).sum(1)[None, :]],
            0,
        ).astype(np.float32)

    aCv, aIv = aug_a(a), aug_a(a0)
    bC_full, bI_full = aug_b(b), aug_b(b0)

    in_maps = []
    for c in range(N_CORES):
        sl = slice(c * O_SHARD, (c + 1) * O_SHARD)
        ab = np.ascontiguousarray(
            np.concatenate([aCv, aIv, bC_full[:, sl], bI_full[:, sl]], axis=1)
        )
        in_maps.append(
            {
                "xT": xT,
                "ab": ab,
                "bias": np.ascontiguousarray(
                    np.concatenate([bias[sl], np.ones(128, np.float32)])
                )[None, :],
            }
        )
    return in_maps


def _run(in_maps, trace=False):
    from concourse.bass_utils import run_bass_kernel_spmd

    if "nc" not in _CACHE:
        _CACHE["nc"] = _build_bass()
    nc = _CACHE["nc"]
    res = run_bass_kernel_spmd(
        nc, in_maps, core_ids=list(range(N_CORES)), trace=trace
    )
    outs = [r["out"] for r in res.results]
    return np.concatenate(outs, axis=1), res


def kernel(**inputs) -> np.ndarray:
    in_maps = _prep_inputs(**inputs)
    out, _ = _run(in_maps, trace=bool(os.environ.get("MESHFC_TRACE")))
    return out


# revision 28
# speedup vs baseline: 1.8001x; 1.8001x over previous
"""MeshFC kernel for 8x TRN2 NeuronCores.

Computes: out = inputs @ w + biases, where
  w[i,o] = ||in_pos[i]-out_pos[o]|| - ||init_in_pos[i]-init_out_pos[o]||

Sharding: tensor-parallel on the output dim (8 x 1024 columns). Each core:
  - generates its weight column block on-chip via the PE using the
    augmented-inner-product identity dist^2 = ||a||^2 - 2 a.b + ||b||^2
    (a single K=7 fp32 matmul per tile), sqrt on ScalarE, subtract on DVE
  - runs the main [4096,2048]x[2048,1024] matmul in float32r (FP22)
Host side: pre-transposes/pre-tiles inputs so every DMA is contiguous,
and concatenates the 8 per-core [4096,1024] outputs.
"""

import os
from contextlib import ExitStack

import numpy as np

NUM_IN, NUM_OUT, SD, BATCH = 2048, 8192, 5, 4096
N_CORES = 8
O_SHARD = NUM_OUT // N_CORES  # 1024
B_TILES = BATCH // 128  # 32
K_TILES = NUM_IN // 128  # 16
O_HALves = O_SHARD // 512  # 2

_CACHE = {}


def _build_bass(variant=""):
    import concourse.bass as bass  # noqa: F401
    import concourse.mybir as mybir
    from concourse import bacc
    from concourse.tile import TileContext

    fp32 = mybir.dt.float32
    fp32r = mybir.dt.float32r

    # Bacc (not plain Bass): its compile() runs generate_event_semaphores +
    # move_matmul_waits_to_ldweights, which split multi-waits that exceed the
    # per-instruction HW sync-wait budget.
    nc = bacc.Bacc("TRN2", name="meshfc")

    xT = nc.dram_tensor("xT", [B_TILES, 128, NUM_IN], fp32r, kind="ExternalInput")
    # packed [aC | aI | bC | bI] along the free axis -> single DMA, single wait
    AB_W = 2 * NUM_IN + 2 * O_SHARD
    ab = nc.dram_tensor("ab", [7, AB_W], fp32, kind="ExternalInput")
    # [bias | ones(128)] packed on one partition
    bias = nc.dram_tensor("bias", [1, O_SHARD + 128], fp32r, kind="ExternalInput")
    out = nc.dram_tensor("out", [BATCH, O_SHARD], fp32, kind="ExternalOutput")

    with ExitStack() as ctx:
        tc = ctx.enter_context(TileContext(nc))
        const = ctx.enter_context(tc.tile_pool(name="const", bufs=1))
        wps = ctx.enter_context(tc.tile_pool(name="wps", bufs=2, space="PSUM"))
        mps = ctx.enter_context(tc.tile_pool(name="mps", bufs=2, space="PSUM"))
        tmp = ctx.enter_context(tc.tile_pool(name="tmp", bufs=2))
        xpool = ctx.enter_context(tc.tile_pool(name="xp", bufs=3))
        opool = ctx.enter_context(tc.tile_pool(name="op", bufs=3))

        # --- constants ---
        ab_sb = const.tile([7, AB_W], fp32, name="ab_sb")
        nc.sync.dma_start(out=ab_sb, in_=ab[:, :])
        aC_sb = ab_sb[:, 0:NUM_IN]
        aI_sb = ab_sb[:, NUM_IN : 2 * NUM_IN]
        bC_sb = ab_sb[:, 2 * NUM_IN : 2 * NUM_IN + O_SHARD]
        bI_sb = ab_sb[:, 2 * NUM_IN + O_SHARD : AB_W]

        # bias lives on one partition; it is added into PSUM via a K=1 matmul
        biasones_sb = const.tile([1, O_SHARD + 128], fp32r, name="biasones_sb")
        nc.sync.dma_start(out=biasones_sb, in_=bias[:, :])
        bias_sb = biasones_sb[:, 0:O_SHARD]
        ones_sb = biasones_sb[:, O_SHARD : O_SHARD + 128]

        # resident weight block: [128, K_TILES, O_SHARD] = 8 MB
        # float32r so the DVE write rounds to FP22 for the fp32r matmul
        w_sb = const.tile([128, K_TILES, O_SHARD], fp32r, name="w_sb")

        # optional on-device repetition for slope timing (variant "repN")
        n_rep = 1
        for tok in variant.split(","):
            if tok.startswith("rep"):
                n_rep = int(tok[3:])

        # --- weight generation ---
        for _rep in range(n_rep):
            _build_body(nc, tc, variant, const, wps, mps, tmp, xpool, opool,
                        aC_sb, aI_sb, bC_sb, bI_sb, bias_sb, ones_sb, w_sb,
                        xT, out, fp32, fp32r)

    nc.finalize()
    return nc


def _build_body(nc, tc, variant, const, wps, mps, tmp, xpool, opool,
                aC_sb, aI_sb, bC_sb, bI_sb, bias_sb, ones_sb, w_sb,
                xT, out, fp32, fp32r):
    import concourse.mybir as mybir  # noqa: F401

    if True:
        if "nowgen" not in variant:
            for kt in range(K_TILES):
                for oh in range(O_HALves):
                    osl = slice(oh * 512, (oh + 1) * 512)
                    psC = wps.tile([128, 512], fp32, tag="psC", bufs=2)
                    psI = wps.tile([128, 512], fp32, tag="psI", bufs=2)
                    nc.tensor.matmul(
                        psC,
                        aC_sb[:, kt * 128 : (kt + 1) * 128],
                        bC_sb[:, osl],
                        start=True,
                        stop=True,
                    )
                    nc.tensor.matmul(
                        psI,
                        aI_sb[:, kt * 128 : (kt + 1) * 128],
                        bI_sb[:, osl],
                        start=True,
                        stop=True,
                    )
                    # clamp dist^2 to >=0 on DVE (HW fp32 rounding can push
                    # the closest pair slightly negative -> sqrt NaN), then
                    # sqrt in place in SBUF. In-place PSUM activation crashes
                    # the exec unit, so everything lands in SBUF tmps.
                    sC = tmp.tile([128, 512], fp32, tag="sC", bufs=2)
                    sI = tmp.tile([128, 512], fp32, tag="sI", bufs=2)
                    nc.vector.tensor_scalar_max(sC, psC, 0.0)
                    nc.vector.tensor_scalar_max(sI, psI, 0.0)
                    nc.scalar.sqrt(sC, sC)
                    nc.scalar.sqrt(sI, sI)
                    nc.vector.tensor_sub(w_sb[:, kt, osl], sC, sI)

        # --- main matmul: out[b,o] = sum_k x[b,k] w[k,o] (+bias) ---
        for bt in range(B_TILES):
            xt = xpool.tile([128, NUM_IN], fp32r, name="xt")
            nc.sync.dma_start(out=xt, in_=xT[bt])
            ot = opool.tile([128, O_SHARD], fp32, name="ot")
            # pre-touch: absorbs the out-DMA slot-release wait on ScalarE so
            # the real drains below stay within the HW sync-wait slot limit
            nc.scalar.mul(ot[0:1, 0:1], ot[0:1, 0:1], 0.0)
            for oh in range(O_HALves):
                osl = slice(oh * 512, (oh + 1) * 512)
                ps = mps.tile([128, 512], fp32, tag="ps", bufs=2)
                for kt in range(K_TILES):
                    nc.tensor.matmul(
                        ps,
                        xt[:, kt * 128 : (kt + 1) * 128],
                        w_sb[:, kt, osl],
                        start=(kt == 0),
                        stop=("nobias" in variant and kt == K_TILES - 1),
                    )
                # += bias (broadcast over rows via rank-1 matmul)
                if "nobias" not in variant:
                    nc.tensor.matmul(
                        ps, ones_sb[:, :], bias_sb[:, osl], start=False, stop=True
                    )
                nc.scalar.copy(ot[:, osl], ps)
            nc.sync.dma_start(out=out[bt * 128 : (bt + 1) * 128, :], in_=ot)


def _prep_inputs(inputs, init_in_pos, init_out_pos, in_pos, out_pos, biases):
    x = np.ascontiguousarray(np.asarray(inputs, dtype=np.float32))
    a = np.asarray(in_pos, dtype=np.float32).reshape(NUM_IN, SD)
    a0 = np.asarray(init_in_pos, dtype=np.float32).reshape(NUM_IN, SD)
    b = np.asarray(out_pos, dtype=np.float32).reshape(NUM_OUT, SD)
    b0 = np.asarray(init_out_pos, dtype=np.float32).reshape(NUM_OUT, SD)
    bias = np.asarray(biases, dtype=np.float32).reshape(NUM_OUT)

    # [bt, p, kt*128+b'] = x[bt*128+b', kt*128+p]
    xT = np.ascontiguousarray(
        x.reshape(B_TILES, 128, K_TILES, 128).transpose(0, 3, 2, 1)
    ).reshape(B_TILES, 128, NUM_IN)

    def aug_a(p):
        return np.concatenate(
            [p.T, (p * p).sum(1)[None, :], np.ones((1, p.shape[0]), np.float32)], 0
        ).astype(np.float32)

    def aug_b(q):
        return np.concatenate(
            [-2.0 * q.T, np.ones((1, q.shape[0]), np.float32), (q * q).sum(1)[None, :]],
            0,
        ).astype(np.float32)

    aCv, aIv = aug_a(a), aug_a(a0)
    bC_full, bI_full = aug_b(b), aug_b(b0)

    in_maps = []
    for c in range(N_CORES):
        sl = slice(c * O_SHARD, (c + 1) * O_SHARD)
        ab = np.ascontiguousarray(
            np.concatenate([aCv, aIv, bC_full[:, sl], bI_full[:, sl]], axis=1)
        )
        in_maps.append(
            {
                "xT": xT,
                "ab": ab,
                "bias": np.ascontiguousarray(
                    np.concatenate([bias[sl], np.ones(128, np.float32)])
                )[None, :],
            }
        )
    return in_maps


def _run(in_maps, trace=False):
    from concourse.bass_utils import run_bass_kernel_spmd

    if "nc" not in _CACHE:
        _CACHE["nc"] = _build_bass()
    nc = _CACHE["nc"]
    res = run_bass_kernel_spmd(
        nc, in_maps, core_ids=list(range(N_CORES)), trace=trace
    )
    outs = [r["out"] for r in res.results]
    return np.concatenate(outs, axis=1), res


def kernel(**inputs) -> np.ndarray:
    in_maps = _prep_inputs(**inputs)
    out, _ = _run(in_maps, trace=bool(os.environ.get("MESHFC_TRACE")))
    return out
